# revision 11
# baseline (speedup 1.0000x reference)
"""Spherical Brownian motion (Stratonovich-Heun, 100 steps) on 8 TRN2 NeuronCores.

Math: per step with unit x and noise u = normal*sqrt_dt:
    a = x.u, b = u.u
    al = 1 + b/2 - a            (= den*alpha, den = 1 + b - a^2 > 0)
    be = 1 + (b - a^2 - a)/2    (= den*beta)
    x' = (al*x + be*u) / |al*x + be*u|
which is algebraically identical to the reference Heun step (verified to
8e-6 absmax in fp32). Noise is generated on host with JAX's threefry
(bit-identical to the reference) and streamed to the device as fp16.

Device layout per core: rows R=524288 as planes [128 partitions, 4096].
State x is fp32-resident in SBUF; per-step compute uses fp16 (DVE 2x mode)
with centered scalars al' = al-1, be' = be-1 so fp16 quantization error
stays ~5e-5. Normalization via fp16 magic-seed + 2 Newton rsqrt iterations
(radial error self-corrects). ScalarE takes the squares, SyncE streams
noise double-buffered.
"""

import numpy as np

N, D = 4194304, 3
NCORES = 8
R = N // NCORES          # 524288 rows per core
P, F = 128, R // 128     # plane [128, 4096]
STEPS = 100

_graph_cache = {}


def build_graph(steps=STEPS, mode="safe"):
    import concourse.bass as bass
    import concourse.mybir as mybir

    key = (steps, mode)
    if key in _graph_cache:
        return _graph_cache[key]

    alu = mybir.AluOpType
    f16, f32, i16 = mybir.dt.float16, mybir.dt.float32, mybir.dt.int16
    ACTF = mybir.ActivationFunctionType

    nc = bass.Bass()
    x_ext = nc.declare_dram_parameter("x", [P, D, F], f32, isOutput=False)
    nz_ext = nc.declare_dram_parameter("nz", [steps, P, D, F], f16, isOutput=False)
    out_ext = nc.declare_dram_parameter("out", [P, D, F], f32, isOutput=True)

    with (
        nc.Block() as block,
        nc.semaphore("dma_sem") as dma_sem,
        nc.semaphore("act_sem") as act_sem,
        nc.semaphore("dve_sem") as dve_sem,
    ):
        S = lambda name, shape, dt: nc.alloc_sbuf_tensor(name, shape, dt).ap()
        xs = S("xs", [P, D, F], f32)       # fp32 state
        x16 = S("x16", [P, D, F], f16)     # fp16 copy of state
        ub = [S("u0", [P, D, F], f16), S("u1", [P, D, F], f16)]
        q = S("q", [P, D, F], f16)         # ACT u-squares, then w
        a = S("a", [P, F], f16)
        b = S("b", [P, F], f16)
        alp = S("alp", [P, F], f16)        # also p1
        bep = S("bep", [P, F], f16)        # also p2
        yv = S("yv", [P, F], f16)          # p3 / a2 / rsqrt ping
        t1 = S("t1", [P, F], f16)
        t2 = S("t2", [P, F], f16)

        # dve_sem increments per step: a-ready (3s+1), bep-ready (3s+2), done (3s+3)
        # act_sem increments per step: q (3s+1), a2 (3s+2), A2+B2 (3s+3)
        # dma_sem: x-load +16, then noise dma s at 16*(s+2)

        @block.sync
        def _(sync: bass.BassEngine):
            sync.dma_start(out=xs[:], in_=x_ext[:]).then_inc(dma_sem, 16)
            sync.dma_start(out=ub[0][:], in_=nz_ext[0]).then_inc(dma_sem, 16)
            if steps > 1:
                sync.dma_start(out=ub[1][:], in_=nz_ext[1]).then_inc(dma_sem, 16)
            for s in range(2, steps):
                sync.wait_ge(dve_sem, 3 * (s - 2) + 3)
                sync.dma_start(out=ub[s % 2][:], in_=nz_ext[s]).then_inc(dma_sem, 16)
            sync.wait_ge(dve_sem, 3 * steps)
            sync.dma_start(out=out_ext[:], in_=xs[:]).then_inc(dma_sem, 16)
            sync.wait_ge(dma_sem, 16 * (steps + 2))

        @block.scalar
        def _(act: bass.BassEngine):
            for s in range(steps):
                u = ub[s % 2]
                act.wait_ge(dma_sem, 16 * (s + 2))
                if s > 0:
                    act.wait_ge(dve_sem, 3 * s)  # prev step done (q holds w until then)
                act.activation(q[:, 0, :], u[:, 0, :], ACTF.Square)
                act.activation(q[:, 1, :], u[:, 1, :], ACTF.Square)
                act.activation(q[:, 2, :], u[:, 2, :], ACTF.Square).then_inc(act_sem, 1)
                act.wait_ge(dve_sem, 3 * s + 1)
                act.activation(yv[:], a[:], ACTF.Square).then_inc(act_sem, 1)  # a2
                # z_c = ytilde16_c^2 into (a, b, alp) planes
                act.wait_ge(dve_sem, 3 * s + 2)
                act.activation(a[:], u[:, 0, :], ACTF.Square)
                act.activation(b[:], u[:, 1, :], ACTF.Square)
                act.activation(alp[:], u[:, 2, :], ACTF.Square).then_inc(act_sem, 1)

        @block.vector
        def _(v: bass.BassEngine):
            TT = v.tensor_tensor
            TS = v.tensor_scalar
            v.wait_ge(dma_sem, 16)
            for s in range(steps):
                u = ub[s % 2]
                u1, u2, u3 = u[:, 0, :], u[:, 1, :], u[:, 2, :]
                x1, x2, x3 = xs[:, 0, :], xs[:, 1, :], xs[:, 2, :]
                h1, h2, h3 = x16[:, 0, :], x16[:, 1, :], x16[:, 2, :]
                w1, w2, w3 = q[:, 0, :], q[:, 1, :], q[:, 2, :]
                v.tensor_copy(h1, x1)
                v.tensor_copy(h2, x2)
                v.tensor_copy(h3, x3)
                v.wait_ge(dma_sem, 16 * (s + 2))
                # products -> a   (p1,p2,p3 live in alp,bep,n2)
                TT(alp[:], h1, u1, alu.mult)
                TT(bep[:], h2, u2, alu.mult)
                TT(yv[:], h3, u3, alu.mult)
                TT(t1[:], alp[:], bep[:], alu.add)
                TT(a[:], t1[:], yv[:], alu.add).then_inc(dve_sem, 1)
                # b from ACT squares
                v.wait_ge(act_sem, 3 * s + 1)
                TT(t1[:], q[:, 0, :], q[:, 1, :], alu.add)
                TT(b[:], t1[:], q[:, 2, :], alu.add)
                # al' = 0.5 b - a
                TS(t2[:], b[:], 0.5, None, alu.mult)
                TT(alp[:], t2[:], a[:], alu.subtract)
                # be' = 0.5 (b - a2 - a)
                v.wait_ge(act_sem, 3 * s + 2)
                TT(t1[:], b[:], yv[:], alu.subtract)
                TT(t2[:], t1[:], a[:], alu.subtract)
                TS(bep[:], t2[:], 0.5, None, alu.mult)
                # w_c = (al'*x_c + u_c) + be'*u_c   (into q planes)
                for (hc, uc, wc) in ((h1, u1, w1), (h2, u2, w2), (h3, u3, w3)):
                    TT(t1[:], alp[:], hc, alu.mult)
                    TT(t2[:], t1[:], uc, alu.add)
                    TT(t1[:], bep[:], uc, alu.mult)
                    TT(wc, t2[:], t1[:], alu.add)
                # ytilde16_c = x_c + w_c  (f16, into the u planes; u is consumed)
                TT(u1, x1, w1, alu.add)
                TT(u2, x2, w2, alu.add)
                TT(u3, x3, w3, alu.add).then_inc(dve_sem, 1)
                # n2 = z1 + z2 + z3 (ACT squares of ytilde16)
                v.wait_ge(act_sem, 3 * s + 3)
                TT(t1[:], a[:], b[:], alu.add)
                TT(t2[:], t1[:], alp[:], alu.add)         # t2 = n2 final
                # rsqrt: fp16 magic seed + 2 newton; seed = 0x59BA - (i>>1)
                #   = ~(i>>1) + 0x59BB: TS(shift,xor) bitwise pair + TS(add)
                TS(t1.bitcast(i16), t2.bitcast(i16), 1, -1, alu.logical_shift_right, alu.bitwise_xor)
                TS(yv.bitcast(i16), t1.bitcast(i16), 22971, None, alu.add)
                ycur, yoth = yv, alp
                for _ in range(2):
                    TT(t1[:], ycur[:], ycur[:], alu.mult)
                    TT(bep[:], t2[:], t1[:], alu.mult)
                    TS(t1[:], bep[:], -0.5, 1.5, alu.mult, alu.add)
                    TT(yoth[:], ycur[:], t1[:], alu.mult)
                    ycur, yoth = yoth, ycur
                # x'_c = (x_c + w_c) * r   (recompute x+w in fp32; the dead
                # ytilde16 planes in the u buffer serve as fp32 scratch)
                H = F // 2
                scrh = [u1.bitcast(f32), u2.bitcast(f32)]
                for c in range(3):
                    for k in range(2):
                        xch = xs[:, c, k * H:(k + 1) * H]
                        wch = q[:, c, k * H:(k + 1) * H]
                        rch = ycur[:, k * H:(k + 1) * H]
                        TT(scrh[k][:], xch, wch, alu.add)
                        ins = TT(xch, scrh[k][:], rch, alu.mult)
                ins.then_inc(dve_sem, 1)

    _graph_cache[key] = nc
    return nc


def _to_planes(arr_rows3, dtype):
    # [N, 3] -> list of 8 per-core [P, 3, F]
    a = arr_rows3.reshape(NCORES, P, F, D).transpose(0, 1, 3, 2)
    return [np.ascontiguousarray(a[c], dtype=dtype) for c in range(NCORES)]


def _from_planes(core_outs):
    # list of 8 [P, 3, F] -> [N, 3]
    a = np.stack(core_outs, axis=0)          # [8, P, 3, F]
    return np.ascontiguousarray(a.transpose(0, 1, 3, 2).reshape(N, D))


def gen_noise(t, steps, chunk=10):
    """Per-core noise [steps, P, 3, F] fp16, bit-matching the reference's
    jax.random stream, pre-scaled by sqrt_dt."""
    import jax

    cpu = jax.devices("cpu")[0]
    sqrt_dt = np.sqrt(np.asarray(float(t) / float(steps), np.float32)).astype(np.float32)
    base_key = jax.random.key(1)

    def gen(i):
        k = jax.random.fold_in(base_key, i)
        nrm = jax.random.normal(k, (N, D), jax.numpy.float32)
        inc = (nrm * sqrt_dt).astype(jax.numpy.float16)
        # [N,3] -> [NCORES, P, F, D] -> [NCORES, P, D, F]
        return inc.reshape(NCORES, P, F, D).transpose(0, 1, 3, 2)

    gen_j = jax.jit(gen, device=cpu)
    out = [np.empty((steps, P, D, F), np.float16) for _ in range(NCORES)]
    for s in range(steps):
        blk = np.asarray(gen_j(s))                   # [NCORES, P, D, F] f16
        for c in range(NCORES):
            out[c][s] = blk[c]
    return out


def kernel(x, t=1, steps=STEPS, mode="safe"):
    from concourse.bass_utils import run_bass_kernel_spmd

    steps = int(steps)
    assert steps == STEPS, f"graph is built for {STEPS} steps, got {steps}"
    x = np.asarray(x, np.float32)
    assert x.shape == (N, D)

    nz = gen_noise(t, steps)
    xp = _to_planes(x, np.float32)
    nc = build_graph(steps, mode)
    in_maps = [{"x": xp[c], "nz": nz[c]} for c in range(NCORES)]
    res = run_bass_kernel_spmd(nc, in_maps, core_ids=list(range(NCORES)))
    return _from_planes([res.results[c]["out"] for c in range(NCORES)])


# revision 17
# speedup vs baseline: 1.8353x; 1.8353x over previous
"""Spherical Brownian motion (Stratonovich-Heun, 100 steps) on 8 TRN2 NeuronCores.

Math: per step with unit x and noise u = normal*sqrt_dt:
    a = x.u, b = u.u
    al = 1 + b/2 - a            (= den*alpha, den = 1 + b - a^2 > 0)
    be = 1 + (b - a^2 - a)/2    (= den*beta)
    x' = (al*x + be*u) / |al*x + be*u|
which is algebraically identical to the reference Heun step (verified to
8e-6 absmax in fp32). Noise is generated on host with JAX's threefry
(bit-identical to the reference) and streamed to the device as fp16.

Device layout per core: rows R=524288 as planes [128 partitions, 4096].
State x is fp32-resident in SBUF; per-step compute uses fp16 (DVE 2x mode)
with centered scalars al' = al-1, be' = be-1 so fp16 quantization error
stays ~5e-5. Normalization via fp16 magic-seed + 2 Newton rsqrt iterations
(radial error self-corrects). ScalarE takes the squares, SyncE streams
noise double-buffered.
"""

import numpy as np

N, D = 4194304, 3
NCORES = 8
R = N // NCORES          # 524288 rows per core
P, F = 128, R // 128     # plane [128, 4096]
STEPS = 100

_graph_cache = {}


def build_graph(steps=STEPS, mode="safe"):
    import concourse.bass as bass
    import concourse.mybir as mybir

    key = (steps, mode)
    if key in _graph_cache:
        return _graph_cache[key]

    alu = mybir.AluOpType
    f16, f32, i16 = mybir.dt.float16, mybir.dt.float32, mybir.dt.int16
    ACTF = mybir.ActivationFunctionType

    if mode == "dve":
        return _build_graph_dve(steps)

    nc = bass.Bass()
    x_ext = nc.declare_dram_parameter("x", [P, D, F], f32, isOutput=False)
    nz_ext = nc.declare_dram_parameter("nz", [steps, P, D, F], f16, isOutput=False)
    out_ext = nc.declare_dram_parameter("out", [P, D, F], f32, isOutput=True)

    with (
        nc.Block() as block,
        nc.semaphore("dma_sem") as dma_sem,
        nc.semaphore("act_sem") as act_sem,
        nc.semaphore("dve_sem") as dve_sem,
    ):
        S = lambda name, shape, dt: nc.alloc_sbuf_tensor(name, shape, dt).ap()
        xs = S("xs", [P, D, F], f32)       # fp32 state
        x16 = S("x16", [P, D, F], f16)     # fp16 copy of state
        ub = [S("u0", [P, D, F], f16), S("u1", [P, D, F], f16)]
        q = S("q", [P, D, F], f16)         # ACT u-squares, then w
        a = S("a", [P, F], f16)
        b = S("b", [P, F], f16)
        alp = S("alp", [P, F], f16)        # also p1
        bep = S("bep", [P, F], f16)        # also p2
        yv = S("yv", [P, F], f16)          # p3 / a2 / rsqrt ping
        t1 = S("t1", [P, F], f16)
        t2 = S("t2", [P, F], f16)

        # dve_sem increments per step: a-ready (3s+1), bep-ready (3s+2), done (3s+3)
        # act_sem increments per step: q (3s+1), a2 (3s+2), A2+B2 (3s+3)
        # dma_sem: x-load +16, then noise dma s at 16*(s+2)

        @block.sync
        def _(sync: bass.BassEngine):
            sync.dma_start(out=xs[:], in_=x_ext[:]).then_inc(dma_sem, 16)
            sync.dma_start(out=ub[0][:], in_=nz_ext[0]).then_inc(dma_sem, 16)
            if steps > 1:
                sync.dma_start(out=ub[1][:], in_=nz_ext[1]).then_inc(dma_sem, 16)
            for s in range(2, steps):
                sync.wait_ge(dve_sem, 3 * (s - 2) + 3)
                sync.dma_start(out=ub[s % 2][:], in_=nz_ext[s]).then_inc(dma_sem, 16)
            sync.wait_ge(dve_sem, 3 * steps)
            sync.dma_start(out=out_ext[:], in_=xs[:]).then_inc(dma_sem, 16)
            sync.wait_ge(dma_sem, 16 * (steps + 2))

        @block.scalar
        def _(act: bass.BassEngine):
            for s in range(steps):
                u = ub[s % 2]
                act.wait_ge(dma_sem, 16 * (s + 2))
                if s > 0:
                    act.wait_ge(dve_sem, 3 * s)  # prev step done (q holds w until then)
                act.activation(q[:, 0, :], u[:, 0, :], ACTF.Square)
                act.activation(q[:, 1, :], u[:, 1, :], ACTF.Square)
                act.activation(q[:, 2, :], u[:, 2, :], ACTF.Square).then_inc(act_sem, 1)
                act.wait_ge(dve_sem, 3 * s + 1)
                act.activation(yv[:], a[:], ACTF.Square).then_inc(act_sem, 1)  # a2
                # z_c = ytilde16_c^2 into (a, b, alp) planes
                act.wait_ge(dve_sem, 3 * s + 2)
                act.activation(a[:], u[:, 0, :], ACTF.Square)
                act.activation(b[:], u[:, 1, :], ACTF.Square)
                act.activation(alp[:], u[:, 2, :], ACTF.Square).then_inc(act_sem, 1)

        @block.vector
        def _(v: bass.BassEngine):
            TT = v.tensor_tensor
            TS = v.tensor_scalar
            v.wait_ge(dma_sem, 16)
            for s in range(steps):
                u = ub[s % 2]
                u1, u2, u3 = u[:, 0, :], u[:, 1, :], u[:, 2, :]
                x1, x2, x3 = xs[:, 0, :], xs[:, 1, :], xs[:, 2, :]
                h1, h2, h3 = x16[:, 0, :], x16[:, 1, :], x16[:, 2, :]
                w1, w2, w3 = q[:, 0, :], q[:, 1, :], q[:, 2, :]
                v.tensor_copy(h1, x1)
                v.tensor_copy(h2, x2)
                v.tensor_copy(h3, x3)
                v.wait_ge(dma_sem, 16 * (s + 2))
                # products -> a   (p1,p2,p3 live in alp,bep,n2)
                TT(alp[:], h1, u1, alu.mult)
                TT(bep[:], h2, u2, alu.mult)
                TT(yv[:], h3, u3, alu.mult)
                TT(t1[:], alp[:], bep[:], alu.add)
                TT(a[:], t1[:], yv[:], alu.add).then_inc(dve_sem, 1)
                # b from ACT squares
                v.wait_ge(act_sem, 3 * s + 1)
                TT(t1[:], q[:, 0, :], q[:, 1, :], alu.add)
                TT(b[:], t1[:], q[:, 2, :], alu.add)
                # al' = 0.5 b - a
                TS(t2[:], b[:], 0.5, None, alu.mult)
                TT(alp[:], t2[:], a[:], alu.subtract)
                # be' = 0.5 (b - a2 - a)
                v.wait_ge(act_sem, 3 * s + 2)
                TT(t1[:], b[:], yv[:], alu.subtract)
                TT(t2[:], t1[:], a[:], alu.subtract)
                TS(bep[:], t2[:], 0.5, None, alu.mult)
                # w_c = (al'*x_c + u_c) + be'*u_c   (into q planes)
                for (hc, uc, wc) in ((h1, u1, w1), (h2, u2, w2), (h3, u3, w3)):
                    TT(t1[:], alp[:], hc, alu.mult)
                    TT(t2[:], t1[:], uc, alu.add)
                    TT(t1[:], bep[:], uc, alu.mult)
                    TT(wc, t2[:], t1[:], alu.add)
                # ytilde16_c = x_c + w_c  (f16, into the u planes; u is consumed)
                TT(u1, x1, w1, alu.add)
                TT(u2, x2, w2, alu.add)
                TT(u3, x3, w3, alu.add).then_inc(dve_sem, 1)
                # n2 = z1 + z2 + z3 (ACT squares of ytilde16)
                v.wait_ge(act_sem, 3 * s + 3)
                TT(t1[:], a[:], b[:], alu.add)
                TT(t2[:], t1[:], alp[:], alu.add)         # t2 = n2 final
                # rsqrt: fp16 magic seed + 2 newton; seed = 0x59BA - (i>>1)
                #   = ~(i>>1) + 0x59BB: TS(shift,xor) bitwise pair + TS(add)
                TS(t1.bitcast(i16), t2.bitcast(i16), 1, -1, alu.logical_shift_right, alu.bitwise_xor)
                TS(yv.bitcast(i16), t1.bitcast(i16), 22971, None, alu.add)
                ycur, yoth = yv, alp
                for _ in range(2):
                    TT(t1[:], ycur[:], ycur[:], alu.mult)
                    TT(bep[:], t2[:], t1[:], alu.mult)
                    TS(t1[:], bep[:], -0.5, 1.5, alu.mult, alu.add)
                    TT(yoth[:], ycur[:], t1[:], alu.mult)
                    ycur, yoth = yoth, ycur
                # x'_c = (x_c + w_c) * r   (recompute x+w in fp32; the dead
                # ytilde16 planes in the u buffer serve as fp32 scratch)
                H = F // 2
                scrh = [u1.bitcast(f32), u2.bitcast(f32)]
                for c in range(3):
                    for k in range(2):
                        xch = xs[:, c, k * H:(k + 1) * H]
                        wch = q[:, c, k * H:(k + 1) * H]
                        rch = ycur[:, k * H:(k + 1) * H]
                        TT(scrh[k][:], xch, wch, alu.add)
                        ins = TT(xch, scrh[k][:], rch, alu.mult)
                ins.then_inc(dve_sem, 1)

    _graph_cache[key] = nc
    return nc


def _build_graph_dve(steps=STEPS):
    """Single-engine (VectorE) variant: no cross-engine stalls. b = |u|^2 is
    precomputed on host and shipped as a 4th noise plane."""
    import concourse.bass as bass
    import concourse.mybir as mybir

    key = (steps, "dve")
    if key in _graph_cache:
        return _graph_cache[key]

    alu = mybir.AluOpType
    f16, f32, i16 = mybir.dt.float16, mybir.dt.float32, mybir.dt.int16

    nc = bass.Bass()
    x_ext = nc.declare_dram_parameter("x", [P, D, F], f32, isOutput=False)
    nz_ext = nc.declare_dram_parameter("nz", [steps, P, D + 1, F], f16, isOutput=False)
    out_ext = nc.declare_dram_parameter("out", [P, D, F], f32, isOutput=True)

    with (
        nc.Block() as block,
        nc.semaphore("dma_sem") as dma_sem,
        nc.semaphore("dve_sem") as dve_sem,
    ):
        S = lambda name, shape, dt: nc.alloc_sbuf_tensor(name, shape, dt).ap()
        xs = S("xs", [P, D, F], f32)
        x16 = S("x16", [P, D, F], f16)
        ub = [S("u0", [P, D + 1, F], f16), S("u1", [P, D + 1, F], f16)]
        q = S("q", [P, D, F], f16)         # w planes
        a = S("a", [P, F], f16)
        alp = S("alp", [P, F], f16)
        bep = S("bep", [P, F], f16)
        yv = S("yv", [P, F], f16)
        t1 = S("t1", [P, F], f16)

        @block.sync
        def _(sync: bass.BassEngine):
            sync.dma_start(out=xs[:], in_=x_ext[:]).then_inc(dma_sem, 16)
            sync.dma_start(out=ub[0][:], in_=nz_ext[0]).then_inc(dma_sem, 16)
            if steps > 1:
                sync.dma_start(out=ub[1][:], in_=nz_ext[1]).then_inc(dma_sem, 16)
            for s in range(2, steps):
                sync.wait_ge(dve_sem, s - 1)
                sync.dma_start(out=ub[s % 2][:], in_=nz_ext[s]).then_inc(dma_sem, 16)
            sync.wait_ge(dve_sem, steps)
            sync.dma_start(out=out_ext[:], in_=xs[:]).then_inc(dma_sem, 16)
            sync.wait_ge(dma_sem, 16 * (steps + 2))

        @block.vector
        def _(v: bass.BassEngine):
            TT = v.tensor_tensor
            TS = v.tensor_scalar
            v.wait_ge(dma_sem, 16)
            H = F // 2
            for s in range(steps):
                u = ub[s % 2]
                u1, u2, u3, b = u[:, 0, :], u[:, 1, :], u[:, 2, :], u[:, 3, :]
                x1, x2, x3 = xs[:, 0, :], xs[:, 1, :], xs[:, 2, :]
                h1, h2, h3 = x16[:, 0, :], x16[:, 1, :], x16[:, 2, :]
                w1, w2, w3 = q[:, 0, :], q[:, 1, :], q[:, 2, :]
                v.tensor_copy(h1, x1)
                v.tensor_copy(h2, x2)
                v.tensor_copy(h3, x3)
                v.wait_ge(dma_sem, 16 * (s + 2))
                # a = sum x_c u_c  (products in alp, bep, yv)
                TT(alp[:], h1, u1, alu.mult)
                TT(bep[:], h2, u2, alu.mult)
                TT(yv[:], h3, u3, alu.mult)
                TT(t1[:], alp[:], bep[:], alu.add)
                TT(a[:], t1[:], yv[:], alu.add)
                # al' = 0.5 b - a
                TS(t1[:], b, 0.5, None, alu.mult)
                TT(alp[:], t1[:], a[:], alu.subtract)
                # be' = 0.5 (b - a^2 - a)
                TT(t1[:], a[:], a[:], alu.mult)
                TT(yv[:], b, t1[:], alu.subtract)
                TT(t1[:], yv[:], a[:], alu.subtract)
                TS(bep[:], t1[:], 0.5, None, alu.mult)
                # w_c = (al'*x_c + u_c) + be'*u_c  (a is dead, serves as scratch)
                for (hc, uc, wc) in ((h1, u1, w1), (h2, u2, w2), (h3, u3, w3)):
                    TT(t1[:], alp[:], hc, alu.mult)
                    TT(a[:], t1[:], uc, alu.add)
                    TT(t1[:], bep[:], uc, alu.mult)
                    TT(wc, a[:], t1[:], alu.add)
                # ytilde16_c = x_c + w_c (into u planes), z_c = ytilde^2
                TT(u1, x1, w1, alu.add)
                TT(u2, x2, w2, alu.add)
                TT(u3, x3, w3, alu.add)
                TT(a[:], u1, u1, alu.mult)
                TT(yv[:], u2, u2, alu.mult)
                TT(alp[:], u3, u3, alu.mult)
                TT(t1[:], a[:], yv[:], alu.add)
                TT(bep[:], t1[:], alp[:], alu.add)        # bep = n2
                # rsqrt
                TS(t1.bitcast(i16), bep.bitcast(i16), 1, -1, alu.logical_shift_right, alu.bitwise_xor)
                TS(yv.bitcast(i16), t1.bitcast(i16), 22971, None, alu.add)
                ycur, yoth = yv, alp
                for _ in range(2):
                    TT(t1[:], ycur[:], ycur[:], alu.mult)
                    TT(a[:], bep[:], t1[:], alu.mult)
                    TS(t1[:], a[:], -0.5, 1.5, alu.mult, alu.add)
                    TT(yoth[:], ycur[:], t1[:], alu.mult)
                    ycur, yoth = yoth, ycur
                # x'_c = (x_c + w_c) * r
                scrh = [u1.bitcast(f32), u2.bitcast(f32)]
                for c in range(3):
                    for k in range(2):
                        xch = xs[:, c, k * H:(k + 1) * H]
                        wch = q[:, c, k * H:(k + 1) * H]
                        rch = ycur[:, k * H:(k + 1) * H]
                        TT(scrh[k][:], xch, wch, alu.add)
                        ins = TT(xch, scrh[k][:], rch, alu.mult)
                ins.then_inc(dve_sem, 1)

    _graph_cache[key] = nc
    return nc


def _to_planes(arr_rows3, dtype):
    # [N, 3] -> list of 8 per-core [P, 3, F]
    a = arr_rows3.reshape(NCORES, P, F, D).transpose(0, 1, 3, 2)
    return [np.ascontiguousarray(a[c], dtype=dtype) for c in range(NCORES)]


def _from_planes(core_outs):
    # list of 8 [P, 3, F] -> [N, 3]
    a = np.stack(core_outs, axis=0)          # [8, P, 3, F]
    return np.ascontiguousarray(a.transpose(0, 1, 3, 2).reshape(N, D))


def gen_noise(t, steps, with_b=False):
    """Per-core noise [steps, P, 3(+1), F] fp16, bit-matching the reference's
    jax.random stream, pre-scaled by sqrt_dt. with_b appends b = |u|^2
    (computed with the same fp16 rounding the device would use)."""
    import jax
    import jax.numpy as jnp

    cpu = jax.devices("cpu")[0]
    sqrt_dt = np.sqrt(np.asarray(float(t) / float(steps), np.float32)).astype(np.float32)
    base_key = jax.random.key(1)
    f16, f32 = jnp.float16, jnp.float32

    def gen(i):
        k = jax.random.fold_in(base_key, i)
        nrm = jax.random.normal(k, (N, D), f32)
        inc = (nrm * sqrt_dt).astype(f16)
        pl = inc.reshape(NCORES, P, F, D).transpose(0, 1, 3, 2)  # [NC,P,D,F]
        if not with_b:
            return pl
        q = (pl.astype(f32) ** 2).astype(f16)
        b = ((q[:, :, 0].astype(f32) + q[:, :, 1].astype(f32)).astype(f16).astype(f32)
             + q[:, :, 2].astype(f32)).astype(f16)
        return jnp.concatenate([pl, b[:, :, None, :]], axis=2)  # [NC,P,4,F]

    gen_j = jax.jit(gen, device=cpu)
    nd = D + 1 if with_b else D
    out = [np.empty((steps, P, nd, F), np.float16) for _ in range(NCORES)]
    for s in range(steps):
        blk = np.asarray(gen_j(s))
        for c in range(NCORES):
            out[c][s] = blk[c]
    return out


def kernel(x, t=1, steps=STEPS, mode="safe"):
    from concourse.bass_utils import run_bass_kernel_spmd

    steps = int(steps)
    assert steps == STEPS, f"graph is built for {STEPS} steps, got {steps}"
    x = np.asarray(x, np.float32)
    assert x.shape == (N, D)

    nz = gen_noise(t, steps, with_b=(mode == "dve"))
    xp = _to_planes(x, np.float32)
    nc = build_graph(steps, mode)
    in_maps = [{"x": xp[c], "nz": nz[c]} for c in range(NCORES)]
    res = run_bass_kernel_spmd(nc, in_maps, core_ids=list(range(NCORES)))
    return _from_planes([res.results[c]["out"] for c in range(NCORES)])
